# revision 2
# baseline (speedup 1.0000x reference)
"""Trainium2 Bass kernel for single-head self-attention (EnhancedSelfAttention).

Reference computation (per batch b):
    q = x @ Wq.T + bq ; k = x @ Wk.T + bk ; v = x @ Wv.T + bv
    out = softmax(q @ k.T / sqrt(D)) @ v

Sharding: 8 cores = 4 batches x 2 query-halves. Each core computes the
attention output for its 1024 queries. The V projection (key-side, shared by
both cores of a batch pair) is NOT duplicated: each core computes v for its
own 1024 keys only and the halves are exchanged through two pipelined 2-rank
AllGather collectives (1 MB each) that overlap with the rT/scores phases.

All key-indexed tensors (xT8 for scores, PT, vv) use GLOBAL key order (no
per-core rotation) so the AllGather concatenation in rank order lands in
SPMD-uniform addresses; the query side uses a per-core slice xTq.

Weight-only preprocessing happens on the host (it is input-independent):
  - softmax over keys is shift-invariant along the key axis, so the bk term
    (constant per query) cancels exactly: bk is never sent to the device.
  - scores[sq,sk] = x[sk,:] . r[sq,:] with r = x_q @ C + u, where
    C = Wq^T @ Wk and u = Wk^T @ bq are computed on the host in f32 and
    shipped bf16/f32. This removes the k-projection entirely.
  - x^T layouts and casts (bf16 own-half slice, fp8 full) are done host-side.

Device phases (fp32 PSUM accumulation everywhere):
  - v[sk, e] = sum_d xTq[d, sk](lhsT) WvT[d, e] + bv[e]  (bf16) for the OWN
    1024 keys only, emitted in two 8-PSUM-group blocks (d2 outermost so the
    in-order Tensor queue consumes WvT/xTq tiles in DMA-arrival order); each
    block's result is bounced to DRAM and AllGather'd with the pair core,
    then read back into vv[16 tiles, global key order] while rT/scores run.
  - rT[d1, sq] = sum_d2 C[d2, d1] xTq[d2, sq] + u[d1]  (bf16), to fp8 SBUF.
  - scores^T[sk, sq] = sum_d xT8[d, sk](lhsT) rT8[d, sq] in fp8-e4m3
    DoubleRow perf mode (2x PE throughput); exp(scores/32) by ScalarE
    straight out of PSUM (no max-shift needed: |scores|/32 < ~3).
  - out[sq, e] = sum_sk PT[sk, sq](lhsT) v[sk, e] split 384+384+(256+denom)
    across three PSUM banks (the appended all-ones 1025th v column makes the
    softmax denominator accumulate inside the third chunk); final division by
    per-partition reciprocal split across VectorE and ScalarE, stores
    alternating both HWDGE queues.
"""

import numpy as np
import ml_dtypes

P = 128
D = 1024
S = 2048
SQ = 1024
ND = D // P     # 8 d-tiles
NSK = S // P    # 16 key tiles (global)
NVK = SQ // P   # 8 own-key tiles
FD = 512        # matmul moving free dim
NQC = SQ // FD  # 2 query chunks
SCALE = 1.0 / 32.0

BF16 = ml_dtypes.bfloat16
FP8 = ml_dtypes.float8_e4m3

_cached = None


def _build():
    from contextlib import ExitStack

    import concourse.mybir as mybir
    import concourse.tile as tile
    from concourse import bacc

    f32 = mybir.dt.float32
    bf16 = mybir.dt.bfloat16
    fp8 = mybir.dt.float8e4
    AF = mybir.ActivationFunctionType
    PM = mybir.MatmulPerfMode

    nc = bacc.Bacc("TRN2", target_bir_lowering=False, debug=False, num_devices=8)

    xTq_d = nc.declare_dram_parameter("xTq", [D, SQ], bf16, isOutput=False)
    xT8_d = nc.declare_dram_parameter("xT8", [D, S], fp8, isOutput=False)
    C_d = nc.declare_dram_parameter("C", [D, D], bf16, isOutput=False)
    WvT_d = nc.declare_dram_parameter("WvT", [D, D], bf16, isOutput=False)
    u_d = nc.declare_dram_parameter("u", [P, ND], f32, isOutput=False)
    bv_d = nc.declare_dram_parameter("bvb", [P, D], bf16, isOutput=False)
    out_d = nc.declare_dram_parameter("out", [SQ, D], f32, isOutput=True)

    with tile.TileContext(nc) as tc, ExitStack() as ctx:
        const = ctx.enter_context(tc.tile_pool(name="const", bufs=1))
        persist = ctx.enter_context(tc.tile_pool(name="persist", bufs=1))
        ostage = ctx.enter_context(tc.tile_pool(name="ostage", bufs=4))
        small = ctx.enter_context(tc.tile_pool(name="small", bufs=4))
        dram = ctx.enter_context(tc.tile_pool(name="dram", bufs=1, space="DRAM"))
        # one PSUM pool for the whole kernel (every tile fits a 2KB bank
        # slot): no pool-close barrier ever stalls a phase transition.
        psumB = ctx.enter_context(
            tc.tile_pool(name="psumB", bufs=8, space="PSUM"))

        FA = 384  # out matmul split: 384 + 384 + (256 + denom column)
        u_sb = const.tile([P, ND], f32)
        bv_sb = const.tile([P, D], bf16)
        warm_l = const.tile([P, P], bf16)
        warm_r = const.tile([P, FD], bf16)
        nc.gpsimd.memset(warm_l, 0.0)
        nc.gpsimd.memset(warm_r, 0.0)

        xTq = persist.tile([P, ND, SQ], bf16)    # x^T own queries == own keys
        xT8 = persist.tile([P, ND, S], fp8)      # x^T full, fp8, GLOBAL order
        Csb = persist.tile([P, ND, D], bf16)     # C    [d2, d1]
        WvT = persist.tile([P, ND, D], bf16)     # Wv^T [d, e]
        rT8 = persist.tile([P, ND, SQ], fp8)     # r^T  [d1, sq] fp8
        vstg = persist.tile([P, NVK, D], bf16)   # own-key v staging
        vv = persist.tile([P, NSK, D + 1], bf16)  # v [sk global, e] + ones col
        PT0 = persist.tile([P, NSK, FD], bf16)   # exp(scores/32), qc=0
        PT1 = persist.tile([P, NSK, FD], bf16)   # exp(scores/32), qc=1
        PTs = [PT0, PT1]

        # AllGather bounce buffers: two 1MB chunks (own keys 0:512, 512:1024)
        agin = [dram.tile([SQ // 2, D], bf16, name=f"agin{i}") for i in range(2)]
        agout = [dram.tile([SQ, D], bf16, name=f"agout{i}") for i in range(2)]

        # ---- loads: two HWDGE queues pull concurrently, v-phase data first.
        # sync: WvT then C; scalar: xTq then xT8. u/bv ride the SWDGE queue.
        nc.gpsimd.dma_start(out=u_sb, in_=u_d[:, :])
        nc.gpsimd.dma_start(out=bv_sb, in_=bv_d[:, :])
        for dt in range(ND):
            nc.sync.dma_start(out=WvT[:, dt, :],
                              in_=WvT_d[dt * P:(dt + 1) * P, :])
            nc.scalar.dma_start(out=xTq[:, dt, :],
                                in_=xTq_d[dt * P:(dt + 1) * P, :])
        for dt in range(ND):
            nc.sync.dma_start(out=Csb[:, dt, :],
                              in_=C_d[dt * P:(dt + 1) * P, :])
            nc.scalar.dma_start(out=xT8[:, dt, :],
                                in_=xT8_d[dt * P:(dt + 1) * P, :])

        nc.vector.memset(vv[:, :, D:D + 1], 1.0)

        # ---- v own half, in two chunk-aligned blocks of 8 PSUM groups ----
        # v[sk, e] = sum_d xTq[d, sk](lhsT) WvT[d, e] + bv.  d2 outermost so
        # the in-order Tensor queue consumes xTq/WvT tiles in DMA-arrival
        # order. Each block covers own-key tiles 4c..4c+3 and feeds one
        # AllGather chunk with the pair core.
        first = True
        for c in range(2):
            groups = [(4 * c + i, e) for i in range(4) for e in range(2)]
            pss = [psumB.tile([P, FD], f32, name=f"psv{i}", tag="psb")
                   for i in range(len(groups))]
            if first:
                # warm-up matmuls on constant tiles: the PE p-state ramps to
                # full clock only after ~3us of continuous work, and the PE
                # idles here waiting for the first DMAs anyway. The first
                # real matmul's start=True re-zeroes the bank.
                for _ in range(8):
                    nc.tensor.matmul(pss[0], warm_l, warm_r,
                                     start=True, stop=True)
                first = False
            for dc in range(ND):
                for (skt, e), ps in zip(groups, pss):
                    nc.tensor.matmul(
                        ps,
                        xTq[:, dc, skt * P:(skt + 1) * P],
                        WvT[:, dc, e * FD:(e + 1) * FD],
                        start=(dc == 0), stop=(dc == ND - 1),
                    )
            for (skt, e), ps in zip(groups, pss):
                nc.vector.tensor_add(
                    out=vstg[:, skt, e * FD:(e + 1) * FD], in0=ps,
                    in1=bv_sb[:, e * FD:(e + 1) * FD])
            # bounce this chunk to DRAM and exchange with the pair core
            for t in range(4):
                skt = 4 * c + t
                nc.sync.dma_start(out=agin[c][t * P:(t + 1) * P, :],
                                  in_=vstg[:, skt, :])
            nc.gpsimd.collective_compute(
                "AllGather",
                mybir.AluOpType.bypass,
                replica_groups=[[0, 1], [2, 3], [4, 5], [6, 7]],
                ins=[agin[c][:, :].opt()],
                outs=[agout[c][:, :].opt()],
            )
            # readback: rank0 shard -> global tiles 4c..4c+3, rank1 shard ->
            # global tiles 8+4c..8+4c+3 (own half comes back too: uniform
            # addressing for both cores of the pair).
            for t in range(4):
                nc.scalar.dma_start(
                    out=vv[:, 4 * c + t, 0:D],
                    in_=agout[c][t * P:(t + 1) * P, :])
            for t in range(4):
                nc.scalar.dma_start(
                    out=vv[:, 8 + 4 * c + t, 0:D],
                    in_=agout[c][(4 + t) * P:(5 + t) * P, :])

        # ---- rT ----
        # rT[d1, sq] = sum_d2 C[d2, d1] * xTq[d2, sq]  (+ u[d1])
        for blk in range(0, ND * NQC, 8):
            groups = [divmod(g, NQC) for g in range(blk, blk + 8)]
            pss = [psumB.tile([P, FD], f32, name=f"psr{i}", tag="psb")
                   for i in range(len(groups))]
            for d2c in range(ND):
                for (d1t, qc), ps in zip(groups, pss):
                    nc.tensor.matmul(
                        ps,
                        Csb[:, d2c, d1t * P:(d1t + 1) * P],
                        xTq[:, d2c, qc * FD:(qc + 1) * FD],
                        start=(d2c == 0), stop=(d2c == ND - 1),
                    )
            for (d1t, qc), ps in zip(groups, pss):
                nc.vector.tensor_scalar_add(
                    rT8[:, d1t, qc * FD:(qc + 1) * FD], ps,
                    u_sb[:, d1t:d1t + 1])

        # ---- scores ----
        # scores^T[sk, sq] = sum_d xT8[d, sk](lhsT) * rT8[d, sq] in fp8-e4m3
        # DoubleRow mode: each matmul contracts K=256 as two stacked
        # 128-blocks (lhsT [128, 2, 128], rhs [128, 2, 512]).
        for qc in range(NQC):
            PT = PTs[qc]
            for skt in range(NSK):
                ps = psumB.tile([P, FD], f32, name="pssc", tag="psb")
                for dc in range(ND // 2):
                    nc.tensor.matmul(
                        ps,
                        xT8[:, 2 * dc:2 * dc + 2, skt * P:(skt + 1) * P],
                        rT8[:, 2 * dc:2 * dc + 2, qc * FD:(qc + 1) * FD],
                        start=(dc == 0), stop=(dc == ND // 2 - 1),
                        perf_mode=PM.DoubleRow,
                    )
                nc.scalar.activation(PT[:, skt, :], ps, AF.Exp, scale=SCALE)

        # ---- attention output ----
        for qc in range(NQC):
            PT = PTs[qc]
            # out[sq, e] = sum_sk PT[sk, sq](lhsT) * v[sk, e]; the ones
            # column of vv makes po2's last column the softmax denominator
            for qt in range(FD // P):  # 4 sq-tiles of 128 per chunk
                po0 = psumB.tile([P, FA], f32, name="po0", tag="psb")
                po1 = psumB.tile([P, FA], f32, name="po1", tag="psb")
                po2 = psumB.tile([P, D - 2 * FA + 1], f32, name="po2",
                                 tag="psb")
                rec = small.tile([P, 1], f32)
                ot0 = ostage.tile([P, FA], f32, tag="ot0")
                ot1 = ostage.tile([P, FA], f32, tag="ot1")
                ot2 = ostage.tile([P, D - 2 * FA], f32, tag="ot2")
                row0 = (qc * 4 + qt) * P
                last = qc == NQC - 1 and qt == FD // P - 1
                if last:
                    # final group: accumulation order over PSUM tiles is
                    # commutative, so run the denominator tile's block
                    # first and the divides of po2/po0 overlap the
                    # remaining matmul blocks — only ot1's divide + store
                    # trail the last matmul.
                    for skt in range(NSK):
                        nc.tensor.matmul(
                            po2, PT[:, skt, qt * P:(qt + 1) * P],
                            vv[:, skt, 2 * FA:D + 1],
                            start=(skt == 0), stop=(skt == NSK - 1))
                    nc.vector.reciprocal(
                        rec, po2[:, D - 2 * FA:D - 2 * FA + 1])
                    nc.vector.tensor_scalar_mul(
                        ot2, po2[:, 0:D - 2 * FA], rec)
                    nc.sync.dma_start(out_d[row0:row0 + P, 2 * FA:D], ot2)
                    for skt in range(NSK):
                        nc.tensor.matmul(
                            po0, PT[:, skt, qt * P:(qt + 1) * P],
                            vv[:, skt, 0:FA],
                            start=(skt == 0), stop=(skt == NSK - 1))
                    nc.vector.tensor_scalar_mul(ot0, po0, rec)
                    nc.sync.dma_start(out_d[row0:row0 + P, 0:FA], ot0)
                    for skt in range(NSK):
                        nc.tensor.matmul(
                            po1, PT[:, skt, qt * P:(qt + 1) * P],
                            vv[:, skt, FA:2 * FA],
                            start=(skt == 0), stop=(skt == NSK - 1))
                    # final divide split across both engines in parallel
                    h = FA // 2
                    nc.vector.tensor_scalar_mul(
                        ot1[:, 0:h], po1[:, 0:h], rec)
                    nc.sync.dma_start(
                        out_d[row0:row0 + P, FA:FA + h], ot1[:, 0:h])
                    nc.scalar.activation(
                        ot1[:, h:FA], po1[:, h:FA], AF.Copy, scale=rec)
                    nc.scalar.dma_start(
                        out_d[row0:row0 + P, FA + h:2 * FA],
                        ot1[:, h:FA])
                    continue
                for skt in range(NSK):
                    w_lhsT = PT[:, skt, qt * P:(qt + 1) * P]
                    nc.tensor.matmul(po0, w_lhsT, vv[:, skt, 0:FA],
                                     start=(skt == 0), stop=(skt == NSK - 1))
                    nc.tensor.matmul(po1, w_lhsT, vv[:, skt, FA:2 * FA],
                                     start=(skt == 0), stop=(skt == NSK - 1))
                    nc.tensor.matmul(po2, w_lhsT, vv[:, skt, 2 * FA:D + 1],
                                     start=(skt == 0), stop=(skt == NSK - 1))
                nc.vector.reciprocal(rec, po2[:, D - 2 * FA:D - 2 * FA + 1])
                # divides split across VectorE (ot0/ot2) and ScalarE (ot1)
                # so the PSUM drains run in parallel; stores alternate
                # between the two HWDGE queues.
                nc.vector.tensor_scalar_mul(ot0, po0, rec)
                nc.sync.dma_start(out_d[row0:row0 + P, 0:FA], ot0)
                nc.scalar.activation(ot1, po1, AF.Copy, scale=rec)
                nc.scalar.dma_start(out_d[row0:row0 + P, FA:2 * FA], ot1)
                nc.vector.tensor_scalar_mul(ot2, po2[:, 0:D - 2 * FA], rec)
                nc.sync.dma_start(out_d[row0:row0 + P, 2 * FA:D], ot2)

    nc.compile()
    return nc


def _get_nc():
    global _cached
    if _cached is None:
        _cached = _build()
    return _cached


def make_in_maps(x, Wq, bq, Wk, Wv, bv):
    # Host-side weight prep (input-independent): C = Wq^T Wk, u = Wk^T bq,
    # transposed/cast layouts for x, Wv, bv.
    C = np.ascontiguousarray(
        (Wq.T.astype(np.float32) @ Wk.astype(np.float32)).astype(BF16))
    WvT = np.ascontiguousarray(Wv.T.astype(BF16))
    u = (Wk.T.astype(np.float32) @ bq.astype(np.float32)).astype(np.float32)
    u_t = np.ascontiguousarray(u.reshape(ND, P).T)
    bvb = np.ascontiguousarray(
        np.broadcast_to(bv.astype(BF16)[None, :], (P, D)))

    in_maps = []
    for core in range(8):
        b, h = divmod(core, 2)
        xTb = x[b].T  # [D, S] global order
        xTb8 = np.ascontiguousarray(xTb.astype(FP8))
        xTq = np.ascontiguousarray(xTb[:, h * SQ:(h + 1) * SQ].astype(BF16))
        in_maps.append(
            {"xTq": xTq, "xT8": xTb8, "C": C, "WvT": WvT, "u": u_t,
             "bvb": bvb})
    return in_maps


def kernel(x, Wq, bq, Wk, bk, Wv, bv):
    from concourse.bass_utils import run_bass_kernel_spmd

    x = np.asarray(x, dtype=np.float32)
    Wq = np.asarray(Wq, dtype=np.float32)
    Wk = np.asarray(Wk, dtype=np.float32)
    Wv = np.asarray(Wv, dtype=np.float32)
    bq = np.asarray(bq, dtype=np.float32)
    bv = np.asarray(bv, dtype=np.float32)

    nc = _get_nc()
    in_maps = make_in_maps(x, Wq, bq, Wk, Wv, bv)
    res = run_bass_kernel_spmd(nc, in_maps, list(range(8)))
    out = np.empty((4, S, D), dtype=np.float32)
    for core in range(8):
        b, h = divmod(core, 2)
        out[b, h * SQ:(h + 1) * SQ, :] = res.results[core]["out"]
    return out


# revision 6
# speedup vs baseline: 1.1247x; 1.1247x over previous
"""Trainium2 Bass kernel for single-head self-attention (EnhancedSelfAttention).

Reference computation (per batch b):
    q = x @ Wq.T + bq ; k = x @ Wk.T + bk ; v = x @ Wv.T + bv
    out = softmax(q @ k.T / sqrt(D)) @ v

Sharding: 8 cores = 4 batches x 2 query-halves. Each core computes the
attention output for its 1024 queries. The V projection (key-side, shared by
both cores of a batch pair) is NOT duplicated: each core computes v for its
own 1024 keys only and the halves are exchanged through two pipelined 2-rank
AllGather collectives (1 MB each) that overlap with the rT/scores phases.

All key-indexed tensors (xT8 for scores, PT, vv) use GLOBAL key order (no
per-core rotation) so the AllGather concatenation in rank order lands in
SPMD-uniform addresses; the query side uses a per-core slice xTq.

Weight-only preprocessing happens on the host (it is input-independent):
  - softmax over keys is shift-invariant along the key axis, so the bk term
    (constant per query) cancels exactly: bk is never sent to the device.
  - scores[sq,sk] = x[sk,:] . r[sq,:] with r = x_q @ C + u, where
    C = Wq^T @ Wk and u = Wk^T @ bq are computed on the host in f32 and
    shipped bf16/f32. This removes the k-projection entirely.
  - x^T layouts and casts (bf16 own-half slice, fp8 full) are done host-side.

Device phases (fp32 PSUM accumulation everywhere):
  - v[sk, e] = sum_d xTq[d, sk](lhsT) WvT[d, e] + bv[e]  (bf16) for the OWN
    1024 keys only, emitted in two 8-PSUM-group blocks (d2 outermost so the
    in-order Tensor queue consumes WvT/xTq tiles in DMA-arrival order); each
    block's result is bounced to DRAM and AllGather'd with the pair core,
    then read back into vv[16 tiles, global key order] while rT/scores run.
  - rT[d1, sq] = sum_d2 C[d2, d1] xTq[d2, sq] + u[d1]  (bf16), to fp8 SBUF.
  - scores^T[sk, sq] = sum_d xT8[d, sk](lhsT) rT8[d, sq] in fp8-e4m3
    DoubleRow perf mode (2x PE throughput); exp(scores/32) by ScalarE
    straight out of PSUM (no max-shift needed: |scores|/32 < ~3).
  - out[sq, e] = sum_sk PT[sk, sq](lhsT) v[sk, e] split 384+384+(256+denom)
    across three PSUM banks (the appended all-ones 1025th v column makes the
    softmax denominator accumulate inside the third chunk); final division by
    per-partition reciprocal split across VectorE and ScalarE, stores
    alternating both HWDGE queues.
"""

import numpy as np
import ml_dtypes

P = 128
D = 1024
S = 2048
SQ = 1024
ND = D // P     # 8 d-tiles
NSK = S // P    # 16 key tiles (global)
NVK = SQ // P   # 8 own-key tiles
FD = 512        # matmul moving free dim
NQC = SQ // FD  # 2 query chunks
SCALE = 1.0 / 32.0

BF16 = ml_dtypes.bfloat16
FP8 = ml_dtypes.float8_e4m3

_cached = None


def _build():
    from contextlib import ExitStack

    import concourse.mybir as mybir
    import concourse.tile as tile
    from concourse import bacc

    f32 = mybir.dt.float32
    bf16 = mybir.dt.bfloat16
    fp8 = mybir.dt.float8e4
    AF = mybir.ActivationFunctionType
    PM = mybir.MatmulPerfMode

    nc = bacc.Bacc("TRN2", target_bir_lowering=False, debug=False, num_devices=8)

    xTq_d = nc.declare_dram_parameter("xTq", [D, SQ], bf16, isOutput=False)
    xT8_d = nc.declare_dram_parameter("xT8", [D, S], fp8, isOutput=False)
    C_d = nc.declare_dram_parameter("C", [D, D], bf16, isOutput=False)
    WvT_d = nc.declare_dram_parameter("WvT", [D, D], bf16, isOutput=False)
    u_d = nc.declare_dram_parameter("u", [P, ND], f32, isOutput=False)
    bv_d = nc.declare_dram_parameter("bvb", [P, D], bf16, isOutput=False)
    out_d = nc.declare_dram_parameter("out", [SQ, D], f32, isOutput=True)

    with tile.TileContext(nc) as tc, ExitStack() as ctx:
        const = ctx.enter_context(tc.tile_pool(name="const", bufs=1))
        persist = ctx.enter_context(tc.tile_pool(name="persist", bufs=1))
        ostage = ctx.enter_context(tc.tile_pool(name="ostage", bufs=4))
        small = ctx.enter_context(tc.tile_pool(name="small", bufs=4))
        dram = ctx.enter_context(tc.tile_pool(name="dram", bufs=1, space="DRAM"))
        # one PSUM pool for the whole kernel (every tile fits a 2KB bank
        # slot): no pool-close barrier ever stalls a phase transition.
        psumB = ctx.enter_context(
            tc.tile_pool(name="psumB", bufs=8, space="PSUM"))

        FA = 384  # out matmul split: 384 + 384 + (256 + denom column)
        u_sb = const.tile([P, ND], f32)
        bv_sb = const.tile([P, D], bf16)
        warm_l = const.tile([P, P], bf16)
        warm_r = const.tile([P, FD], bf16)
        nc.gpsimd.memset(warm_l, 0.0)
        nc.gpsimd.memset(warm_r, 0.0)

        xTq = persist.tile([P, ND, SQ], bf16)    # x^T own queries == own keys
        xT8 = persist.tile([P, ND, S], fp8)      # x^T full, fp8, GLOBAL order
        Csb = persist.tile([P, ND, D], bf16)     # C    [d2, d1]
        WvT = persist.tile([P, ND, D], bf16)     # Wv^T [d, e]
        rT8 = persist.tile([P, ND, SQ], fp8)     # r^T  [d1, sq] fp8
        vstg = persist.tile([P, NVK, D], bf16)   # own-key v staging
        vv = persist.tile([P, NSK, D + 1], bf16)  # v [sk global, e] + ones col
        PT0 = persist.tile([P, NSK, FD], bf16)   # exp(scores/32), qc=0
        PT1 = persist.tile([P, NSK, FD], bf16)   # exp(scores/32), qc=1
        PTs = [PT0, PT1]

        # AllGather bounce buffers: two 1MB chunks (own keys 0:512, 512:1024)
        agin = [dram.tile([SQ // 2, D], bf16, name=f"agin{i}") for i in range(2)]
        agout = [dram.tile([SQ, D], bf16, name=f"agout{i}") for i in range(2)]

        # ---- loads: two HWDGE queues pull concurrently, v-phase data first
        # (xTq/WvT d-tiles interleaved across BOTH queues so each dc pass's
        # pair lands together), then C/xT8. bv rides scalar first (needed by
        # the first v drain); u rides SWDGE (tiny, needed only by rT drains).
        # The sync queue later carries the AllGather bounces/readbacks (they
        # block its FIFO while waiting, so C/xT8's sync-half is emitted
        # between bounce batches); ScalarE's queue must stay wait-free or the
        # scores exps queue behind it.
        nc.gpsimd.dma_start(out=u_sb, in_=u_d[:, :])
        nc.scalar.dma_start(out=bv_sb, in_=bv_d[:, :])
        for dt in range(ND):
            qa, qb = (nc.sync, nc.scalar) if dt % 2 == 0 else (nc.scalar, nc.sync)
            qa.dma_start(out=WvT[:, dt, :],
                         in_=WvT_d[dt * P:(dt + 1) * P, :])
            qb.dma_start(out=xTq[:, dt, :],
                         in_=xTq_d[dt * P:(dt + 1) * P, :])
        # scalar halves of C/xT8 (no waits ahead of them on that queue)
        for dt in range(ND):
            q = nc.scalar if dt % 2 == 1 else None
            if q is not None:
                q.dma_start(out=Csb[:, dt, :],
                            in_=C_d[dt * P:(dt + 1) * P, :])
        for dt in range(ND):
            if dt % 2 == 0:
                nc.scalar.dma_start(out=xT8[:, dt, :],
                                    in_=xT8_d[dt * P:(dt + 1) * P, :])

        nc.vector.memset(vv[:, :, D:D + 1], 1.0)

        # ---- v own half, in two chunk-aligned blocks of 8 PSUM groups ----
        # v[sk, e] = sum_d xTq[d, sk](lhsT) WvT[d, e] + bv.  d2 outermost so
        # the in-order Tensor queue consumes xTq/WvT tiles in DMA-arrival
        # order. Each block covers own-key tiles 4c..4c+3 and feeds one
        # AllGather chunk with the pair core.
        first = True
        for c in range(2):
            groups = [(4 * c + i, e) for i in range(4) for e in range(2)]
            pss = [psumB.tile([P, FD], f32, name=f"psv{i}", tag="psb")
                   for i in range(len(groups))]
            if first:
                # warm-up matmuls on constant tiles: the PE p-state ramps to
                # full clock only after ~3us of continuous work, and the PE
                # idles here waiting for the first DMAs anyway. The first
                # real matmul's start=True re-zeroes the bank.
                for _ in range(8):
                    nc.tensor.matmul(pss[0], warm_l, warm_r,
                                     start=True, stop=True)
                first = False
            for dc in range(ND):
                for (skt, e), ps in zip(groups, pss):
                    nc.tensor.matmul(
                        ps,
                        xTq[:, dc, skt * P:(skt + 1) * P],
                        WvT[:, dc, e * FD:(e + 1) * FD],
                        start=(dc == 0), stop=(dc == ND - 1),
                    )
            for (skt, e), ps in zip(groups, pss):
                nc.vector.tensor_add(
                    out=vstg[:, skt, e * FD:(e + 1) * FD], in0=ps,
                    in1=bv_sb[:, e * FD:(e + 1) * FD])
            # bounce this chunk to DRAM and exchange with the pair core
            for t in range(4):
                skt = 4 * c + t
                nc.sync.dma_start(out=agin[c][t * P:(t + 1) * P, :],
                                  in_=vstg[:, skt, :])
            nc.gpsimd.collective_compute(
                "AllGather",
                mybir.AluOpType.bypass,
                replica_groups=[[0, 1], [2, 3], [4, 5], [6, 7]],
                ins=[agin[c][:, :].opt()],
                outs=[agout[c][:, :].opt()],
            )
            # sync-queue halves of the C/xT8 loads ride between the bounce
            # batches (after chunk 0's bounce) / after chunk 1's bounce, so
            # they aren't blocked by the bounce's drain-wait yet still beat
            # the rT/scores phases.
            for dt in range(ND):
                if dt % 2 == c:
                    src, dst = (C_d, Csb) if c == 0 else (xT8_d, xT8)
                    nc.sync.dma_start(out=dst[:, dt, :],
                                      in_=src[dt * P:(dt + 1) * P, :])

        # readback (sync queue: it is idle while waiting on the collectives;
        # ScalarE must stay free for the scores exps): rank0 shard -> global
        # tiles 4c..4c+3, rank1 shard -> global tiles 8+4c..8+4c+3 (own half
        # comes back too: uniform addressing for both cores of the pair).
        for c in range(2):
            for t in range(4):
                nc.sync.dma_start(
                    out=vv[:, 4 * c + t, 0:D],
                    in_=agout[c][t * P:(t + 1) * P, :])
            for t in range(4):
                nc.sync.dma_start(
                    out=vv[:, 8 + 4 * c + t, 0:D],
                    in_=agout[c][(4 + t) * P:(5 + t) * P, :])

        # ---- rT ----
        # rT[d1, sq] = sum_d2 C[d2, d1] * xTq[d2, sq]  (+ u[d1])
        for blk in range(0, ND * NQC, 8):
            groups = [divmod(g, NQC) for g in range(blk, blk + 8)]
            pss = [psumB.tile([P, FD], f32, name=f"psr{i}", tag="psb")
                   for i in range(len(groups))]
            for d2c in range(ND):
                for (d1t, qc), ps in zip(groups, pss):
                    nc.tensor.matmul(
                        ps,
                        Csb[:, d2c, d1t * P:(d1t + 1) * P],
                        xTq[:, d2c, qc * FD:(qc + 1) * FD],
                        start=(d2c == 0), stop=(d2c == ND - 1),
                    )
            for (d1t, qc), ps in zip(groups, pss):
                nc.vector.tensor_scalar_add(
                    rT8[:, d1t, qc * FD:(qc + 1) * FD], ps,
                    u_sb[:, d1t:d1t + 1])

        # ---- scores ----
        # scores^T[sk, sq] = sum_d xT8[d, sk](lhsT) * rT8[d, sq] in fp8-e4m3
        # DoubleRow mode: each matmul contracts K=256 as two stacked
        # 128-blocks (lhsT [128, 2, 128], rhs [128, 2, 512]).
        for qc in range(NQC):
            PT = PTs[qc]
            for skt in range(NSK):
                ps = psumB.tile([P, FD], f32, name="pssc", tag="psb")
                for dc in range(ND // 2):
                    nc.tensor.matmul(
                        ps,
                        xT8[:, 2 * dc:2 * dc + 2, skt * P:(skt + 1) * P],
                        rT8[:, 2 * dc:2 * dc + 2, qc * FD:(qc + 1) * FD],
                        start=(dc == 0), stop=(dc == ND // 2 - 1),
                        perf_mode=PM.DoubleRow,
                    )
                nc.scalar.activation(PT[:, skt, :], ps, AF.Exp, scale=SCALE)

        # ---- attention output ----
        # skt accumulation order puts exchange-chunk-A tiles (global 0-3 and
        # 8-11) first so the first out groups can run before chunk B's
        # readback tiles have all landed.
        SKT_ORDER = [0, 1, 2, 3, 8, 9, 10, 11, 4, 5, 6, 7, 12, 13, 14, 15]
        for qc in range(NQC):
            PT = PTs[qc]
            # out[sq, e] = sum_sk PT[sk, sq](lhsT) * v[sk, e]; the ones
            # column of vv makes po2's last column the softmax denominator
            for qt in range(FD // P):  # 4 sq-tiles of 128 per chunk
                po0 = psumB.tile([P, FA], f32, name="po0", tag="psb")
                po1 = psumB.tile([P, FA], f32, name="po1", tag="psb")
                po2 = psumB.tile([P, D - 2 * FA + 1], f32, name="po2",
                                 tag="psb")
                rec = small.tile([P, 1], f32)
                ot0 = ostage.tile([P, FA], f32, tag="ot0")
                ot1 = ostage.tile([P, FA], f32, tag="ot1")
                ot2 = ostage.tile([P, D - 2 * FA], f32, tag="ot2")
                row0 = (qc * 4 + qt) * P
                last = qc == NQC - 1 and qt == FD // P - 1
                if last:
                    # final group: accumulation order over PSUM tiles is
                    # commutative, so run the denominator tile's block
                    # first and the divides of po2/po0 overlap the
                    # remaining matmul blocks — only ot1's divide + store
                    # trail the last matmul.
                    for i, skt in enumerate(SKT_ORDER):
                        nc.tensor.matmul(
                            po2, PT[:, skt, qt * P:(qt + 1) * P],
                            vv[:, skt, 2 * FA:D + 1],
                            start=(i == 0), stop=(i == NSK - 1))
                    nc.vector.reciprocal(
                        rec, po2[:, D - 2 * FA:D - 2 * FA + 1])
                    nc.vector.tensor_scalar_mul(
                        ot2, po2[:, 0:D - 2 * FA], rec)
                    nc.sync.dma_start(out_d[row0:row0 + P, 2 * FA:D], ot2)
                    for i, skt in enumerate(SKT_ORDER):
                        nc.tensor.matmul(
                            po0, PT[:, skt, qt * P:(qt + 1) * P],
                            vv[:, skt, 0:FA],
                            start=(i == 0), stop=(i == NSK - 1))
                    nc.vector.tensor_scalar_mul(ot0, po0, rec)
                    nc.sync.dma_start(out_d[row0:row0 + P, 0:FA], ot0)
                    for i, skt in enumerate(SKT_ORDER):
                        nc.tensor.matmul(
                            po1, PT[:, skt, qt * P:(qt + 1) * P],
                            vv[:, skt, FA:2 * FA],
                            start=(i == 0), stop=(i == NSK - 1))
                    # final divide split across both engines in parallel
                    h = FA // 2
                    nc.vector.tensor_scalar_mul(
                        ot1[:, 0:h], po1[:, 0:h], rec)
                    nc.sync.dma_start(
                        out_d[row0:row0 + P, FA:FA + h], ot1[:, 0:h])
                    nc.scalar.activation(
                        ot1[:, h:FA], po1[:, h:FA], AF.Copy, scale=rec)
                    nc.scalar.dma_start(
                        out_d[row0:row0 + P, FA + h:2 * FA],
                        ot1[:, h:FA])
                    continue
                for i, skt in enumerate(SKT_ORDER):
                    w_lhsT = PT[:, skt, qt * P:(qt + 1) * P]
                    nc.tensor.matmul(po0, w_lhsT, vv[:, skt, 0:FA],
                                     start=(i == 0), stop=(i == NSK - 1))
                    nc.tensor.matmul(po1, w_lhsT, vv[:, skt, FA:2 * FA],
                                     start=(i == 0), stop=(i == NSK - 1))
                    nc.tensor.matmul(po2, w_lhsT, vv[:, skt, 2 * FA:D + 1],
                                     start=(i == 0), stop=(i == NSK - 1))
                nc.vector.reciprocal(rec, po2[:, D - 2 * FA:D - 2 * FA + 1])
                # divides split across VectorE (ot0/ot2) and ScalarE (ot1)
                # so the PSUM drains run in parallel; stores alternate
                # between the two HWDGE queues.
                nc.vector.tensor_scalar_mul(ot0, po0, rec)
                nc.sync.dma_start(out_d[row0:row0 + P, 0:FA], ot0)
                nc.scalar.activation(ot1, po1, AF.Copy, scale=rec)
                nc.scalar.dma_start(out_d[row0:row0 + P, FA:2 * FA], ot1)
                nc.vector.tensor_scalar_mul(ot2, po2[:, 0:D - 2 * FA], rec)
                nc.sync.dma_start(out_d[row0:row0 + P, 2 * FA:D], ot2)

    nc.compile()
    return nc


def _get_nc():
    global _cached
    if _cached is None:
        _cached = _build()
    return _cached


def make_in_maps(x, Wq, bq, Wk, Wv, bv):
    # Host-side weight prep (input-independent): C = Wq^T Wk, u = Wk^T bq,
    # transposed/cast layouts for x, Wv, bv.
    C = np.ascontiguousarray(
        (Wq.T.astype(np.float32) @ Wk.astype(np.float32)).astype(BF16))
    WvT = np.ascontiguousarray(Wv.T.astype(BF16))
    u = (Wk.T.astype(np.float32) @ bq.astype(np.float32)).astype(np.float32)
    u_t = np.ascontiguousarray(u.reshape(ND, P).T)
    bvb = np.ascontiguousarray(
        np.broadcast_to(bv.astype(BF16)[None, :], (P, D)))

    in_maps = []
    for core in range(8):
        b, h = divmod(core, 2)
        xTb = x[b].T  # [D, S] global order
        xTb8 = np.ascontiguousarray(xTb.astype(FP8))
        xTq = np.ascontiguousarray(xTb[:, h * SQ:(h + 1) * SQ].astype(BF16))
        in_maps.append(
            {"xTq": xTq, "xT8": xTb8, "C": C, "WvT": WvT, "u": u_t,
             "bvb": bvb})
    return in_maps


def kernel(x, Wq, bq, Wk, bk, Wv, bv):
    from concourse.bass_utils import run_bass_kernel_spmd

    x = np.asarray(x, dtype=np.float32)
    Wq = np.asarray(Wq, dtype=np.float32)
    Wk = np.asarray(Wk, dtype=np.float32)
    Wv = np.asarray(Wv, dtype=np.float32)
    bq = np.asarray(bq, dtype=np.float32)
    bv = np.asarray(bv, dtype=np.float32)

    nc = _get_nc()
    in_maps = make_in_maps(x, Wq, bq, Wk, Wv, bv)
    res = run_bass_kernel_spmd(nc, in_maps, list(range(8)))
    out = np.empty((4, S, D), dtype=np.float32)
    for core in range(8):
        b, h = divmod(core, 2)
        out[b, h * SQ:(h + 1) * SQ, :] = res.results[core]["out"]
    return out
